# revision 5
# baseline (speedup 1.0000x reference)
"""Trainium2 Bass kernel for multi-head attention (b=4, n=2048, d=512, h=8, dk=dv=64).

Sharding: 8 cores = 4 batches x 2 query-halves. Each core computes K/V for its
full batch sequence (2048) and attention outputs for its 1024 query rows.
No collectives needed; host stacks the per-core [1024, 512] outputs.

Per-core dataflow (f32r matmuls; PV in bf16):
  x^T [512, 2048] staged in SBUF.
  Head-PAIR packed projections: Q^T/K^T computed per pair p (heads 2p, 2p+1):
    lhsT = w[:, ch, p*128:(p+1)*128] -> out [128 rows = headA 64 dims | headB
    64 dims, n].  Head A lives on partitions 0:64, head B on 64:128.
  ST matmul contracts K=64 directly (no block-diagonal build, no replication):
    lhsT = kt[rows, jc*128:+128] [64, 128j], rhs = qt[rows, i-slice] [64, 512]
    -> out S^T [128 j, 512 i].  Odd heads use partitions 64:128
    (tile_position row 64).  Costs the same PE cycles as K=128.
  V   = x Wv  (+ ones col) [per j-chunk: 128j, 8h*65] in bf16
  P^T = exp(S^T) (bias folded into Q; no max-subtraction: logits < ~50)
  PV accumulated over all 16 j-chunks into [65, 512] per (head, i-half);
    row 64 = denominator.  Normalized via reciprocal + partition broadcast.
  y = out^T.T @ Wo + bo (accumulate over head pairs), DMA out per 128 rows.

PSUM budget (8 banks): st pool 2 bufs x [128, 2jc, 512] f32 (2 banks each)
+ qk pool 3 bufs x [128, 512] (1 bank each) + pv 1 buf [65, 512] (1 bank).
"""
import numpy as np

B, N, MODEL = 4, 2048, 512
H, DK = 8, 64
SCALE = DK ** -0.5
NP = H // 2         # head pairs
NI = 1024           # query rows per core
NCH = MODEL // 128  # model-dim chunks
NJC = N // 128      # key/value chunks
JB = 2              # j-chunks per ST/exp batch
NB = NJC // JB      # batches per (head, i-half)

_COMPILED = None


def _build():
    import concourse.bass as bass
    from concourse import bacc
    import concourse.mybir as mybir
    import concourse.tile as tile

    F32 = mybir.dt.float32
    F32R = mybir.dt.float32r
    BF16 = mybir.dt.bfloat16
    EXP = mybir.ActivationFunctionType.Exp

    nc = bacc.Bacc("TRN2", target_bir_lowering=False, debug=False, num_devices=8)
    xt_in = nc.dram_tensor("xt", [MODEL, N], F32R, kind="ExternalInput")
    wq_in = nc.dram_tensor("wq", [MODEL, MODEL], F32R, kind="ExternalInput")
    wk_in = nc.dram_tensor("wk", [MODEL, MODEL], F32R, kind="ExternalInput")
    wv_in = nc.dram_tensor("wv", [MODEL, MODEL], F32R, kind="ExternalInput")
    relb_in = nc.dram_tensor("relb", [128, NP], F32, kind="ExternalInput")
    wo_in = nc.dram_tensor("wo", [MODEL, MODEL], F32R, kind="ExternalInput")
    bo_in = nc.dram_tensor("bo", [1, MODEL], F32, kind="ExternalInput")
    onesb_in = nc.dram_tensor("onesb", [128, NJC * H], BF16, kind="ExternalInput")
    y_out = nc.dram_tensor("y", [NI, MODEL], F32, kind="ExternalOutput")

    with tile.TileContext(nc) as tc:
        with (
            tc.tile_pool(name="w", bufs=1) as wp,
            tc.tile_pool(name="acts", bufs=1) as ap,
            tc.tile_pool(name="st", bufs=2, space="PSUM") as stp,
            tc.tile_pool(name="qk", bufs=3, space="PSUM") as qkp,
            tc.tile_pool(name="pv", bufs=1, space="PSUM") as pvp,
        ):
            # ---------- persistent tiles ----------
            wq = wp.tile([128, NCH, MODEL], F32R, tag="wq")
            wk = wp.tile([128, NCH, MODEL], F32R, tag="wk")
            wv = wp.tile([128, NCH, MODEL], F32R, tag="wv")
            wo = wp.tile([128, NP, MODEL], F32R, tag="wo")
            relb = wp.tile([128, NP], F32, tag="relb")
            bo = wp.tile([1, MODEL], F32, tag="bo")
            bo_b = wp.tile([128, MODEL], F32, tag="bo_b")
            onesb_t = wp.tile([128, NJC * H], BF16, tag="onesb")

            xt0 = ap.tile([128, NCH, 512], F32R, tag="xt0")
            xt1 = ap.tile([128, NCH, 512], F32R, tag="xt1")
            xt2 = ap.tile([128, NCH, 512], F32R, tag="xt2")
            xt3 = ap.tile([128, NCH, 512], F32R, tag="xt3")
            xts = [xt0, xt1, xt2, xt3]
            vv_a = ap.tile([128, NJC // 2, H * 65], BF16, tag="vva")
            vv_b = ap.tile([128, NJC // 2, H * 65], BF16, tag="vvb")
            qt_t0 = ap.tile([128, NI], F32R, tag="qt0")
            qt_t1 = ap.tile([128, NI], F32R, tag="qt1")
            kt_t0 = ap.tile([128, NJC * 128], F32R, tag="kt0")
            kt_t1 = ap.tile([128, NJC * 128], F32R, tag="kt1")
            outt = ap.tile([128, NP, NI], F32R, tag="outt")

            def vvt(jc):
                return (vv_a if jc < NJC // 2 else vv_b)[:, jc % (NJC // 2)]

            def r3(d):
                return d[:].rearrange("(c p) n -> p c n", p=128)

            def xtv(ch, start, size):
                t = xts[start // 512]
                off = start % 512
                assert off + size <= 512
                return t[:, ch, off:off + size]

            # ---------- input staging, priority order ----------
            dma_engs = (nc.sync, nc.gpsimd, nc.scalar)
            _dma_i = [0]

            def dma(out, in_):
                dma_engs[_dma_i[0] % len(dma_engs)].dma_start(out=out, in_=in_)
                _dma_i[0] += 1

            xsrc = r3(xt_in)
            # wq first (Q proj of pair 0), then x for own queries (i 0:1024)
            for ch in range(NCH):
                dma(wq[:, ch], r3(wq_in)[:, ch])
            dma(relb[:], relb_in[:])
            for q in range(2):
                for chh in range(2):
                    dma(xts[q][:, chh * 2:(chh + 1) * 2, :],
                        xsrc[:, chh * 2:(chh + 1) * 2, q * 512:(q + 1) * 512])
            for ch in range(NCH):
                dma(wk[:, ch], r3(wk_in)[:, ch])
            for q in range(2, 4):
                for chh in range(2):
                    dma(xts[q][:, chh * 2:(chh + 1) * 2, :],
                        xsrc[:, chh * 2:(chh + 1) * 2, q * 512:(q + 1) * 512])
            for ch in range(NCH):
                dma(wv[:, ch], r3(wv_in)[:, ch])
            dma(onesb_t[:], onesb_in[:])
            for ch in range(NCH):
                dma(wo[:, ch], r3(wo_in)[:, ch])
            dma(bo[:], bo_in[:])
            nc.gpsimd.partition_broadcast(bo_b[:], bo[:])
            # ones columns of V_aug: contiguous DMA to scratch, strided DVE copy
            for vh in range(2):
                nc.vector.tensor_copy(
                    (vv_a if vh == 0 else vv_b)[:]
                    .rearrange("p j (h e) -> p (j h) e", e=65)[:, :, 64:65],
                    onesb_t[:, vh * NJC * H // 2:(vh + 1) * NJC * H // 2]
                    .rearrange("p (n o) -> p n o", o=1))

            with (
                tc.tile_pool(name="pt", bufs=4) as ptp,
                tc.tile_pool(name="norm", bufs=2) as np_,
                tc.tile_pool(name="ysb", bufs=2) as yp_sb,
            ):
                # ---- head-pair packed Q^T / K^T projections ----
                def emit_qk(p):
                    qt = qt_t0 if p % 2 == 0 else qt_t1
                    kt = kt_t0 if p % 2 == 0 else kt_t1
                    cols = slice(p * 128, (p + 1) * 128)
                    for g in range(2):
                        q_ps = qkp.tile([128, 512], F32, tag="qk")
                        for ch in range(NCH):
                            nc.tensor.matmul(
                                q_ps[:], wq[:, ch, cols], xtv(ch, g * 512, 512),
                                start=(ch == 0), stop=(ch == NCH - 1))
                        nc.vector.tensor_scalar_add(
                            qt[:, g * 512:(g + 1) * 512], q_ps[:],
                            relb[:, p:p + 1])
                    for g in range(4):
                        k_ps = qkp.tile([128, 512], F32, tag="qk")
                        for ch in range(NCH):
                            nc.tensor.matmul(
                                k_ps[:], wk[:, ch, cols], xtv(ch, g * 512, 512),
                                start=(ch == 0), stop=(ch == NCH - 1))
                        nc.vector.tensor_copy(
                            kt[:, g * 512:(g + 1) * 512], k_ps[:])

                def emit_v(bi):
                    for jc in range(bi * JB * 2, (bi + 1) * JB * 2):
                        v_ps = qkp.tile([128, 512], F32, tag="qk")
                        for ch in range(NCH):
                            nc.tensor.matmul(
                                v_ps[:], xtv(ch, jc * 128, 128), wv[:, ch],
                                start=(ch == 0), stop=(ch == NCH - 1))
                        nc.vector.tensor_copy(
                            vvt(jc).rearrange("p (h e) -> p h e", e=65)[:, :, 0:64],
                            v_ps[:].rearrange("p (h e) -> p h e", e=64))

                emit_qk(0)

                for h in range(H):
                    hp, hr = h // 2, (h % 2) * 64
                    qt = qt_t0 if hp % 2 == 0 else qt_t1
                    kt = kt_t0 if hp % 2 == 0 else kt_t1
                    rows = slice(hr, hr + 64)
                    for ih in range(2):
                        pv_t = pvp.tile([65, 512], F32, tag="pv")
                        isl = slice(ih * 512, (ih + 1) * 512)
                        for bi in range(NB):
                            st = stp.tile([128, JB * 512], F32, tag="st")
                            for k in range(JB):
                                jc = bi * JB + k
                                nc.tensor.matmul(
                                    st[:, k * 512:(k + 1) * 512],
                                    kt[rows, jc * 128:(jc + 1) * 128],
                                    qt[rows, isl], start=True, stop=True)
                            pt = ptp.tile([128, JB * 512], BF16, tag="pt")
                            nc.scalar.activation(pt[:], st[:], EXP, scale=1.0)
                            # interleave V projection during head 0's first half
                            if h == 0 and ih == 0 and bi < 4:
                                emit_v(bi)
                            if ih == 0 and bi == 0 and h % 2 == 0 and h + 2 < H:
                                emit_qk(hp + 1)
                            for k in range(JB):
                                jc = bi * JB + k
                                nc.tensor.matmul(
                                    pv_t[:],
                                    vvt(jc)[:, h * 65:(h + 1) * 65],
                                    pt[:, k * 512:(k + 1) * 512],
                                    start=(jc == 0), stop=(jc == NJC - 1))
                        den = np_.tile([1, 512], F32, tag="den")
                        nc.vector.tensor_copy(den[:], pv_t[64:65, :])
                        rrow = np_.tile([1, 512], F32, tag="rrow")
                        nc.vector.reciprocal_approx_fast(rrow[:], den[:])
                        rb = np_.tile([64, 512], F32, tag="rb")
                        nc.gpsimd.partition_broadcast(rb[:], rrow[:])
                        nc.vector.tensor_tensor(
                            out=outt[rows, hp, isl],
                            in0=pv_t[0:64, :], in1=rb[:],
                            op=mybir.AluOpType.mult)

                # ---------- output projection ----------
                for ib in range(NI // 128):
                    y_ps = qkp.tile([128, 512], F32, tag="qk")
                    for hp2 in range(NP):
                        nc.tensor.matmul(
                            y_ps[:], outt[:, hp2, ib * 128:(ib + 1) * 128],
                            wo[:, hp2], start=(hp2 == 0), stop=(hp2 == NP - 1))
                    y_sb = yp_sb.tile([128, MODEL], F32, tag="ysb")
                    nc.vector.tensor_tensor(out=y_sb[:], in0=y_ps[:], in1=bo_b[:],
                                            op=mybir.AluOpType.add)
                    nc.sync.dma_start(out=y_out[ib * 128:(ib + 1) * 128, :],
                                      in_=y_sb[:])

    nc.compile()
    return nc


def _get_compiled():
    global _COMPILED
    if _COMPILED is None:
        _COMPILED = _build()
    return _COMPILED


def kernel(x, Wq, Wk, Wv, Wo, bo, rel_content_bias, _trace=False):
    from concourse.bass_utils import run_bass_kernel_spmd
    import ml_dtypes

    nc = _get_compiled()

    x = np.asarray(x, dtype=np.float32)
    Wq = np.asarray(Wq, dtype=np.float32)
    Wk = np.asarray(Wk, dtype=np.float32)
    Wv = np.asarray(Wv, dtype=np.float32)
    Wo = np.asarray(Wo, dtype=np.float32)
    bo = np.asarray(bo, dtype=np.float32)
    bias = np.asarray(rel_content_bias, dtype=np.float32).reshape(H, DK)

    Wq_s = (Wq * SCALE).astype(np.float32)
    # relb column p = [bias of head 2p (64) | bias of head 2p+1 (64)]
    relb = bias.reshape(NP, 2 * DK).T.astype(np.float32)  # [128, NP]
    onesb = np.ones((128, NJC * H), ml_dtypes.bfloat16)
    shared = {"wq": Wq_s, "wk": Wk, "wv": Wv, "relb": relb, "wo": Wo,
              "bo": bo[None, :], "onesb": onesb}

    in_maps = []
    for c in range(8):
        b, half = c // 2, c % 2
        xt = np.ascontiguousarray(x[b].T)              # [512, 2048]
        if half:
            xt = np.ascontiguousarray(np.roll(xt, -NI, axis=1))
        in_maps.append({"xt": xt, **shared})

    res = run_bass_kernel_spmd(nc, in_maps, core_ids=list(range(8)),
                               trace=_trace)
    out = np.empty((B, N, MODEL), np.float32)
    for c in range(8):
        b, half = c // 2, c % 2
        out[b, half * NI:(half + 1) * NI, :] = res.results[c]["y"]
    if _trace:
        return out, res
    return out
